# revision 7
# baseline (speedup 1.0000x reference)
"""GAT layer kernel for Trainium2, sharded across 8 NeuronCores.

Strategy (per sharding hint): row-shard the 8192x8192 attention matrix —
core c owns query rows [c*1024, (c+1)*1024). Each core:
  Phase A: computes the full h = X @ W (8192x256, replicated) on PE, with
           e_r = h @ a_r fused in as an extra output column, plus a ones
           column appended for the softmax denominator.
  Phase B: computes e_l for its own 1024 queries (tiny matmuls), transposes
           it to a row, and broadcasts across partitions.
  Phase C: for each 128-key tile: builds unnormalized attention
           p[k, q] = exp(leakyrelu(e_l[q] + e_r[k])) * adj[q, k] directly in
           [key, query] layout (exp(leaky(t)) == max(exp(t), exp(0.2 t))),
           then accumulates h_prime^T contributions via PE matmuls
           out[q, :] += p[:, q].T @ [h | 1].  The ones column yields the
           softmax denominator for free; the final [128, 256] tile is scaled
           by its reciprocal.

Host-side prep is layout only: X transposed, adj transposed+packed to uint8
per-core slices, and the attention projection folded into the weight matrix
(waug = [W | W @ a_r], wl = W @ a_l).
"""

import numpy as np
from contextlib import ExitStack

N = 8192
IN_DIM = 512
D = 256
CORES = 8
QL = N // CORES          # queries per core
KT = N // 128            # 64 key tiles
QT = QL // 128           # 8 query tiles per core
KK = IN_DIM // 128       # 4 contraction tiles for h
HS = D + 2               # h storage stride: [h(256) | ones | pad] (fp32r needs even free sizes)
ALPHA = 0.2

_CACHE = {}


def _build_program(el_imm, er_imm, has_bias, reps=None):
    """Build + compile the Bass program. el_imm/er_imm are scalar offsets
    folded into e_l / e_r (a_bias + b@a_l, b@a_r). has_bias adds h += b."""
    import concourse.mybir as mybir
    import concourse.tile as tile
    from concourse import bacc
    from concourse.masks import make_identity

    f32 = mybir.dt.float32
    f32r = mybir.dt.float32r
    u8 = mybir.dt.uint8
    AF = mybir.ActivationFunctionType
    ALU = mybir.AluOpType

    nc = bacc.Bacc(trn_type="TRN2", debug=False)

    xT = nc.dram_tensor("xT", [IN_DIM, N], f32r, kind="ExternalInput")
    xTq = nc.dram_tensor("xTq", [IN_DIM, QL], f32r, kind="ExternalInput")
    waug = nc.dram_tensor("waug", [IN_DIM, HS], f32r, kind="ExternalInput")
    wl = nc.dram_tensor("wl", [IN_DIM, 2], f32r, kind="ExternalInput")
    adjT = nc.dram_tensor("adjT", [N, QL], u8, kind="ExternalInput")
    brow = nc.dram_tensor("brow", [1, D], f32, kind="ExternalInput")
    hp = nc.dram_tensor("hp", [QL, D], f32, kind="ExternalOutput")

    with tile.TileContext(nc) as tc, ExitStack() as ctx:
        if reps is not None:
            ctx.enter_context(tc.For_i(0, reps, 1))
        const = ctx.enter_context(tc.tile_pool(name="const", bufs=1))

        ident = const.tile([128, 128], f32)
        make_identity(nc, ident[:])

        # waug [512, 257] -> sbuf [128, (kk, 257)]
        waug_sb = const.tile([128, KK * HS], f32r)
        nc.sync.dma_start(
            waug_sb[:].rearrange("p (k n) -> p k n", n=HS),
            waug.ap().rearrange("(k p) n -> p k n", p=128),
        )
        # wl [512, 1] -> sbuf [128, 4]
        wl_sb = const.tile([128, KK * 2], f32r)
        nc.sync.dma_start(
            wl_sb[:].rearrange("p (k n) -> p k n", n=2),
            wl.ap().rearrange("(k p) n -> p k n", p=128),
        )
        # xTq [512, 1024] -> sbuf [128, (kk, 1024)]
        xq = const.tile([128, KK * QL], f32r)
        nc.sync.dma_start(
            xq[:].rearrange("p (k n) -> p k n", n=QL),
            xTq.ap().rearrange("(k p) n -> p k n", p=128),
        )
        if has_bias:
            brow_sb = const.tile([1, D], f32)
            nc.sync.dma_start(brow_sb[:1, :], brow.ap())
            b_bcast = const.tile([128, D], f32)
            nc.gpsimd.partition_broadcast(b_bcast[:], brow_sb[:1, :])

        # persistent state
        h_sb = const.tile([128, KT * HS], f32r)      # [h | 1] per key tile
        er_sb = const.tile([128, KT], f32)
        er02_sb = const.tile([128, KT], f32)
        el_own = const.tile([128, QT], f32)
        el_row = const.tile([1, QL], f32)
        el_bcast = const.tile([128, QL], f32)

        # ones columns of h storage (via f32 const copy: memset on f32r is
        # rejected by walrus, and the f32r matmul input needs a "rounding"
        # producer instruction)
        ones_src = const.tile([128, KT], f32)
        nc.vector.memset(ones_src[:], 1.0)
        zeros_src = const.tile([128, KT], f32)
        nc.vector.memset(zeros_src[:], 0.0)
        hv = h_sb[:].rearrange("p (k j) -> p k j", j=HS)
        nc.vector.tensor_copy(hv[:, :, D : D + 1], ones_src[:].rearrange("p (k n) -> p k n", n=1))
        nc.vector.tensor_copy(hv[:, :, D + 1 : D + 2], zeros_src[:].rearrange("p (k n) -> p k n", n=1))

        # ---- Phase B part 1: e_l for this core's queries (tiny matmuls) ----
        with tc.tile_pool(name="psel", bufs=1, space="PSUM") as psel_pool:
            ps_el = psel_pool.tile([128, 2 * QT], f32)
            xqv = xq[:].rearrange("p (k n) -> p k n", n=QL)
            for t in range(QT):
                for kk in range(KK):
                    nc.tensor.matmul(
                        ps_el[:, 2 * t : 2 * t + 2],
                        xqv[:, kk, t * 128 : (t + 1) * 128],
                        wl_sb[:, 2 * kk : 2 * kk + 2],
                        start=(kk == 0),
                        stop=(kk == KK - 1),
                    )
            nc.vector.tensor_copy(
                el_own[:], ps_el[:].rearrange("p (t j) -> p t j", j=2)[:, :, 0]
            )

        with tc.tile_pool(name="pselr", bufs=1, space="PSUM") as pselr_pool:
            el_row_ps = pselr_pool.tile([1, QL], f32)
            for t in range(QT):
                nc.tensor.transpose(
                    el_row_ps[0:1, t * 128 : (t + 1) * 128],
                    el_own[:, t : t + 1],
                    ident[:],
                )
            nc.vector.tensor_scalar_add(el_row[:1, :], el_row_ps[0:1, :], float(el_imm))
        nc.gpsimd.partition_broadcast(el_bcast[:], el_row[:1, :])

        # ---- Phase A: h = X @ waug over all 64 node tiles ----
        xTv = xT.ap().rearrange("(k p) n -> p k n", p=128)
        NCH = 8            # node tiles per x chunk
        with (
            tc.tile_pool(name="xch", bufs=2) as xch_pool,
            tc.tile_pool(name="psA", bufs=4, space="PSUM") as psA_pool,
        ):
            for ch in range(KT // NCH):
                xch = xch_pool.tile([128, KK * NCH * 128], f32r)
                nc.sync.dma_start(
                    xch[:].rearrange("p (k n) -> p k n", n=NCH * 128),
                    xTv[:, :, ch * NCH * 128 : (ch + 1) * NCH * 128],
                )
                xchv = xch[:].rearrange("p (k n) -> p k n", n=NCH * 128)
                for t in range(NCH):
                    i = ch * NCH + t
                    psA = psA_pool.tile([128, HS], f32)
                    for kk in range(KK):
                        nc.tensor.matmul(
                            psA[:],
                            xchv[:, kk, t * 128 : (t + 1) * 128],
                            waug_sb[:, kk * HS : (kk + 1) * HS],
                            start=(kk == 0),
                            stop=(kk == KK - 1),
                        )
                    if has_bias:
                        nc.vector.tensor_tensor(
                            h_sb[:, i * HS : i * HS + D], psA[:, 0:D], b_bcast[:],
                            op=ALU.add,
                        )
                    else:
                        nc.vector.tensor_copy(h_sb[:, i * HS : i * HS + D], psA[:, 0:D])
                    nc.vector.tensor_scalar_add(
                        er_sb[:, i : i + 1], psA[:, D : D + 1], float(er_imm)
                    )
        nc.vector.tensor_scalar_mul(er02_sb[:], er_sb[:], ALPHA)

        # ---- Phase C: masked attention + h_prime accumulation ----
        with (
            tc.tile_pool(name="psC", bufs=1, space="PSUM") as psC_pool,
            tc.tile_pool(name="adjp", bufs=4) as adj_pool,
            tc.tile_pool(name="up", bufs=2) as u_pool,
            tc.tile_pool(name="vp", bufs=2) as v_pool,
            tc.tile_pool(name="sp", bufs=2) as s_pool,
            tc.tile_pool(name="pp", bufs=3) as p_pool,
            tc.tile_pool(name="op", bufs=2) as o_pool,
        ):
            acc = [
                psC_pool.tile([128, HS], f32, name=f"acc{jj}", tag=f"acc{jj}")
                for jj in range(QT)
            ]
            for k in range(KT):
                adj_t = adj_pool.tile([128, QL], u8)
                nc.sync.dma_start(adj_t[:], adjT.ap()[k * 128 : (k + 1) * 128, :])
                u_t = u_pool.tile([128, QL], f32)
                nc.scalar.activation(
                    u_t[:], el_bcast[:], AF.Exp, bias=er_sb[:, k : k + 1]
                )
                v_t = v_pool.tile([128, QL], f32)
                nc.scalar.activation(
                    v_t[:], el_bcast[:], AF.Exp, bias=er02_sb[:, k : k + 1],
                    scale=ALPHA,
                )
                s_t = s_pool.tile([128, QL], f32)
                nc.vector.tensor_tensor(s_t[:], u_t[:], v_t[:], op=ALU.max)
                p_t = p_pool.tile([128, QL], f32r)
                nc.vector.tensor_tensor(p_t[:], s_t[:], adj_t[:], op=ALU.mult)
                hk = h_sb[:, k * HS : (k + 1) * HS]
                for jj in range(QT):
                    nc.tensor.matmul(
                        acc[jj][:],
                        p_t[:, jj * 128 : (jj + 1) * 128],
                        hk,
                        start=(k == 0),
                        stop=(k == KT - 1),
                    )
            for jj in range(QT):
                recip = o_pool.tile([128, 1], f32, tag="recip")
                nc.vector.reciprocal(recip[:], acc[jj][:, D : D + 1])
                o_t = o_pool.tile([128, D], f32, tag="out")
                nc.vector.tensor_scalar_mul(o_t[:], acc[jj][:, 0:D], recip[:])
                nc.sync.dma_start(hp.ap()[jj * 128 : (jj + 1) * 128, :], o_t[:])

    nc.compile()
    return nc


def _get_program(el_imm, er_imm, has_bias, reps=None):
    key = (round(float(el_imm), 12), round(float(er_imm), 12), bool(has_bias), reps)
    if key not in _CACHE:
        _CACHE[key] = _build_program(el_imm, er_imm, has_bias, reps=reps)
    return _CACHE[key]


def _run(nc, in_maps):
    from concourse.bass_utils import run_bass_kernel_spmd

    res = run_bass_kernel_spmd(nc, in_maps, core_ids=list(range(CORES)))
    return res.results


def _prep_inputs(adj_matrix, node_features, W, b, a, a_bias):
    X = np.asarray(node_features, dtype=np.float32)
    W = np.asarray(W, dtype=np.float32)
    b = np.asarray(b, dtype=np.float32)
    a = np.asarray(a, dtype=np.float32)
    a_l, a_r = a[:D], a[D:]

    wl = np.ascontiguousarray(
        np.concatenate([(W @ a_l)[:, None], np.zeros((IN_DIM, 1), np.float32)], axis=1)
    )
    waug = np.ascontiguousarray(
        np.concatenate([W, (W @ a_r)[:, None], np.zeros((IN_DIM, 1), np.float32)], axis=1)
    )
    xT = np.ascontiguousarray(X.T)
    adjT_u8 = (np.asarray(adj_matrix) > 0).T.astype(np.uint8)
    adjT_u8 = np.ascontiguousarray(adjT_u8)

    has_bias = bool(np.any(b != 0.0))
    el_imm = float(a_bias) + float(b @ a_l)
    er_imm = float(b @ a_r)
    brow = np.ascontiguousarray(b.reshape(1, D))

    in_maps = []
    for c in range(CORES):
        qr = slice(c * QL, (c + 1) * QL)
        in_maps.append(
            {
                "xT": xT,
                "xTq": np.ascontiguousarray(xT[:, qr]),
                "waug": waug,
                "wl": wl,
                "adjT": np.ascontiguousarray(adjT_u8[:, qr]),
                "brow": brow,
            }
        )
    return in_maps, el_imm, er_imm, has_bias


def kernel(**inputs):
    adj_matrix = inputs["adj_matrix"]
    node_features = inputs["node_features"]
    W = inputs["W"]
    b = inputs["b"]
    a = inputs["a"]
    a_bias = inputs["a_bias"]

    in_maps, el_imm, er_imm, has_bias = _prep_inputs(
        adj_matrix, node_features, W, b, a, a_bias
    )
    nc = _get_program(el_imm, er_imm, has_bias)
    results = _run(nc, in_maps)
    out = np.concatenate([results[c]["hp"] for c in range(CORES)], axis=0)
    return out.astype(np.float32)


if __name__ == "__main__":
    rng = np.random.default_rng(0)
    demo = {
        "adj_matrix": rng.integers(0, 2, (N, N)).astype(np.int32),
        "node_features": rng.standard_normal((N, IN_DIM), dtype=np.float32),
        "W": rng.standard_normal((IN_DIM, D), dtype=np.float32) * 0.05,
        "b": np.zeros(D, np.float32),
        "a": rng.standard_normal(2 * D, dtype=np.float32) * 0.1,
        "a_bias": np.zeros((), np.float32),
    }
    out = kernel(**demo)
    print(out.shape, out.dtype)


# revision 9
# speedup vs baseline: 1.0530x; 1.0530x over previous
"""GAT layer kernel for Trainium2, sharded across 8 NeuronCores.

Strategy (per sharding hint): row-shard the 8192x8192 attention matrix —
core c owns query rows [c*1024, (c+1)*1024). Each core:
  Phase A: computes the full h = X @ W (8192x256, replicated) on PE, with
           e_r = h @ a_r fused in as an extra output column, plus a ones
           column appended for the softmax denominator.
  Phase B: computes e_l for its own 1024 queries (tiny matmuls), transposes
           it to a row, and broadcasts across partitions.
  Phase C: for each 128-key tile: builds unnormalized attention
           p[k, q] = exp(leakyrelu(e_l[q] + e_r[k])) * adj[q, k] directly in
           [key, query] layout (exp(leaky(t)) == max(exp(t), exp(0.2 t))),
           then accumulates h_prime^T contributions via PE matmuls
           out[q, :] += p[:, q].T @ [h | 1].  The ones column yields the
           softmax denominator for free; the final [128, 256] tile is scaled
           by its reciprocal.

Host-side prep is layout only: X transposed, adj transposed+packed to uint8
per-core slices, and the attention projection folded into the weight matrix
(waug = [W | W @ a_r], wl = W @ a_l).
"""

import numpy as np
from contextlib import ExitStack

N = 8192
IN_DIM = 512
D = 256
CORES = 8
QL = N // CORES          # queries per core
KT = N // 128            # 64 key tiles
QT = QL // 128           # 8 query tiles per core
KK = IN_DIM // 128       # 4 contraction tiles for h
HS = D + 2               # h storage stride: [h(256) | ones | pad] (fp32r needs even free sizes)
ALPHA = 0.2

_CACHE = {}


def _build_program(el_imm, er_imm, has_bias, reps=None, use_lrelu=False, pool_mult=False, ew_bf16=True, cast_pool=False):
    """Build + compile the Bass program. el_imm/er_imm are scalar offsets
    folded into e_l / e_r (a_bias + b@a_l, b@a_r). has_bias adds h += b."""
    import concourse.mybir as mybir
    import concourse.tile as tile
    from concourse import bacc
    from concourse.masks import make_identity

    f32 = mybir.dt.float32
    f32r = mybir.dt.float32r
    bf16 = mybir.dt.bfloat16
    u8 = mybir.dt.uint8
    AF = mybir.ActivationFunctionType
    ALU = mybir.AluOpType

    nc = bacc.Bacc(trn_type="TRN2", debug=False)

    xT = nc.dram_tensor("xT", [IN_DIM, N], f32r, kind="ExternalInput")
    xTq = nc.dram_tensor("xTq", [IN_DIM, QL], f32r, kind="ExternalInput")
    waug = nc.dram_tensor("waug", [IN_DIM, HS], f32r, kind="ExternalInput")
    wl = nc.dram_tensor("wl", [IN_DIM, 2], f32r, kind="ExternalInput")
    adjT = nc.dram_tensor("adjT", [N, QL], u8, kind="ExternalInput")
    brow = nc.dram_tensor("brow", [1, D], f32, kind="ExternalInput")
    hp = nc.dram_tensor("hp", [QL, D], f32, kind="ExternalOutput")

    with tile.TileContext(nc) as tc, ExitStack() as ctx:
        if reps is not None:
            ctx.enter_context(tc.For_i(0, reps, 1))
        const = ctx.enter_context(tc.tile_pool(name="const", bufs=1))

        ident = const.tile([128, 128], f32)
        make_identity(nc, ident[:])

        # waug [512, 257] -> sbuf [128, (kk, 257)]
        waug_sb = const.tile([128, KK * HS], f32r)
        nc.sync.dma_start(
            waug_sb[:].rearrange("p (k n) -> p k n", n=HS),
            waug.ap().rearrange("(k p) n -> p k n", p=128),
        )
        # wl [512, 1] -> sbuf [128, 4]
        wl_sb = const.tile([128, KK * 2], f32r)
        nc.sync.dma_start(
            wl_sb[:].rearrange("p (k n) -> p k n", n=2),
            wl.ap().rearrange("(k p) n -> p k n", p=128),
        )
        # xTq [512, 1024] -> sbuf [128, (kk, 1024)]
        xq = const.tile([128, KK * QL], f32r)
        nc.sync.dma_start(
            xq[:].rearrange("p (k n) -> p k n", n=QL),
            xTq.ap().rearrange("(k p) n -> p k n", p=128),
        )
        if has_bias:
            brow_sb = const.tile([1, D], f32)
            nc.sync.dma_start(brow_sb[:1, :], brow.ap())
            b_bcast = const.tile([128, D], f32)
            nc.gpsimd.partition_broadcast(b_bcast[:], brow_sb[:1, :])

        # persistent state
        hdt = bf16 if ew_bf16 else f32r
        h_sb = const.tile([128, KT * HS], hdt)      # [h | 1] per key tile
        er_sb = const.tile([128, KT], f32)
        er02_sb = const.tile([128, KT], f32)
        el_own = const.tile([128, QT], f32)
        el_row = const.tile([1, QL], f32)
        el_bcast = const.tile([128, QL], f32)

        # ones columns of h storage (via f32 const copy: memset on f32r is
        # rejected by walrus, and the f32r matmul input needs a "rounding"
        # producer instruction)
        ones_src = const.tile([128, KT], f32)
        nc.vector.memset(ones_src[:], 1.0)
        zeros_src = const.tile([128, KT], f32)
        nc.vector.memset(zeros_src[:], 0.0)
        hv = h_sb[:].rearrange("p (k j) -> p k j", j=HS)
        nc.vector.tensor_copy(hv[:, :, D : D + 1], ones_src[:].rearrange("p (k n) -> p k n", n=1))
        nc.vector.tensor_copy(hv[:, :, D + 1 : D + 2], zeros_src[:].rearrange("p (k n) -> p k n", n=1))

        # ---- Phase B part 1: e_l for this core's queries (tiny matmuls) ----
        with tc.tile_pool(name="psel", bufs=1, space="PSUM") as psel_pool:
            ps_el = psel_pool.tile([128, 2 * QT], f32)
            xqv = xq[:].rearrange("p (k n) -> p k n", n=QL)
            for t in range(QT):
                for kk in range(KK):
                    nc.tensor.matmul(
                        ps_el[:, 2 * t : 2 * t + 2],
                        xqv[:, kk, t * 128 : (t + 1) * 128],
                        wl_sb[:, 2 * kk : 2 * kk + 2],
                        start=(kk == 0),
                        stop=(kk == KK - 1),
                    )
            nc.vector.tensor_copy(
                el_own[:], ps_el[:].rearrange("p (t j) -> p t j", j=2)[:, :, 0]
            )

        with tc.tile_pool(name="pselr", bufs=1, space="PSUM") as pselr_pool:
            el_row_ps = pselr_pool.tile([1, QL], f32)
            for t in range(QT):
                nc.tensor.transpose(
                    el_row_ps[0:1, t * 128 : (t + 1) * 128],
                    el_own[:, t : t + 1],
                    ident[:],
                )
            nc.vector.tensor_scalar_add(el_row[:1, :], el_row_ps[0:1, :], float(el_imm))
        nc.gpsimd.partition_broadcast(el_bcast[:], el_row[:1, :])

        # ---- Phase A: h = X @ waug over all 64 node tiles ----
        xTv = xT.ap().rearrange("(k p) n -> p k n", p=128)
        NCH = 8            # node tiles per x chunk
        with (
            tc.tile_pool(name="xch", bufs=2) as xch_pool,
            tc.tile_pool(name="psA", bufs=4, space="PSUM") as psA_pool,
        ):
            for ch in range(KT // NCH):
                xch = xch_pool.tile([128, KK * NCH * 128], f32r)
                nc.sync.dma_start(
                    xch[:].rearrange("p (k n) -> p k n", n=NCH * 128),
                    xTv[:, :, ch * NCH * 128 : (ch + 1) * NCH * 128],
                )
                xchv = xch[:].rearrange("p (k n) -> p k n", n=NCH * 128)
                for t in range(NCH):
                    i = ch * NCH + t
                    psA = psA_pool.tile([128, HS], f32)
                    for kk in range(KK):
                        nc.tensor.matmul(
                            psA[:],
                            xchv[:, kk, t * 128 : (t + 1) * 128],
                            waug_sb[:, kk * HS : (kk + 1) * HS],
                            start=(kk == 0),
                            stop=(kk == KK - 1),
                        )
                    if has_bias:
                        nc.vector.tensor_tensor(
                            h_sb[:, i * HS : i * HS + D], psA[:, 0:D], b_bcast[:],
                            op=ALU.add,
                        )
                    else:
                        nc.vector.tensor_copy(h_sb[:, i * HS : i * HS + D], psA[:, 0:D])
                    nc.vector.tensor_scalar_add(
                        er_sb[:, i : i + 1], psA[:, D : D + 1], float(er_imm)
                    )
        nc.vector.tensor_scalar_mul(er02_sb[:], er_sb[:], ALPHA)

        # ---- Phase C: masked attention + h_prime accumulation ----
        with (
            tc.tile_pool(name="psC", bufs=1, space="PSUM") as psC_pool,
            tc.tile_pool(name="adjp", bufs=4) as adj_pool,
            tc.tile_pool(name="up", bufs=2) as u_pool,
            tc.tile_pool(name="vp", bufs=2) as v_pool,
            tc.tile_pool(name="sp", bufs=2) as s_pool,
            tc.tile_pool(name="pp", bufs=3) as p_pool,
            tc.tile_pool(name="abp", bufs=2) as adjb_pool,
            tc.tile_pool(name="op", bufs=2) as o_pool,
        ):
            acc = [
                psC_pool.tile([128, HS], f32, name=f"acc{jj}", tag=f"acc{jj}")
                for jj in range(QT)
            ]
            ewdt = bf16 if ew_bf16 else f32
            for k in range(KT):
                adj_t = adj_pool.tile([128, QL], u8)
                nc.sync.dma_start(adj_t[:], adjT.ap()[k * 128 : (k + 1) * 128, :])
                u_t = u_pool.tile([128, QL], ewdt)
                nc.scalar.activation(
                    u_t[:], el_bcast[:], AF.Exp, bias=er_sb[:, k : k + 1]
                )
                v_t = v_pool.tile([128, QL], ewdt)
                nc.scalar.activation(
                    v_t[:], el_bcast[:], AF.Exp, bias=er02_sb[:, k : k + 1],
                    scale=ALPHA,
                )
                s_t = s_pool.tile([128, QL], ewdt)
                nc.vector.tensor_tensor(s_t[:], u_t[:], v_t[:], op=ALU.max)
                p_t = p_pool.tile([128, QL], bf16 if ew_bf16 else f32r)
                if ew_bf16:
                    adjb = adjb_pool.tile([128, QL], bf16)
                    if cast_pool:
                        nc.gpsimd.tensor_copy(adjb[:], adj_t[:])
                    else:
                        nc.vector.tensor_copy(adjb[:], adj_t[:])
                    nc.vector.tensor_tensor(p_t[:], s_t[:], adjb[:], op=ALU.mult)
                else:
                    nc.vector.tensor_tensor(p_t[:], s_t[:], adj_t[:], op=ALU.mult)
                hk = h_sb[:, k * HS : (k + 1) * HS]
                for jj in range(QT):
                    nc.tensor.matmul(
                        acc[jj][:],
                        p_t[:, jj * 128 : (jj + 1) * 128],
                        hk,
                        start=(k == 0),
                        stop=(k == KT - 1),
                    )
            for jj in range(QT):
                recip = o_pool.tile([128, 1], f32, tag="recip")
                nc.vector.reciprocal(recip[:], acc[jj][:, D : D + 1])
                o_t = o_pool.tile([128, D], f32, tag="out")
                nc.vector.tensor_scalar_mul(o_t[:], acc[jj][:, 0:D], recip[:])
                nc.sync.dma_start(hp.ap()[jj * 128 : (jj + 1) * 128, :], o_t[:])

    nc.compile()
    return nc


def _get_program(el_imm, er_imm, has_bias, reps=None, **kw):
    key = (round(float(el_imm), 12), round(float(er_imm), 12), bool(has_bias), reps,
           tuple(sorted(kw.items())))
    if key not in _CACHE:
        _CACHE[key] = _build_program(el_imm, er_imm, has_bias, reps=reps, **kw)
    return _CACHE[key]


def _run(nc, in_maps):
    from concourse.bass_utils import run_bass_kernel_spmd

    res = run_bass_kernel_spmd(nc, in_maps, core_ids=list(range(CORES)))
    return res.results


def _prep_inputs(adj_matrix, node_features, W, b, a, a_bias):
    X = np.asarray(node_features, dtype=np.float32)
    W = np.asarray(W, dtype=np.float32)
    b = np.asarray(b, dtype=np.float32)
    a = np.asarray(a, dtype=np.float32)
    a_l, a_r = a[:D], a[D:]

    wl = np.ascontiguousarray(
        np.concatenate([(W @ a_l)[:, None], np.zeros((IN_DIM, 1), np.float32)], axis=1)
    )
    waug = np.ascontiguousarray(
        np.concatenate([W, (W @ a_r)[:, None], np.zeros((IN_DIM, 1), np.float32)], axis=1)
    )
    xT = np.ascontiguousarray(X.T)
    adjT_u8 = (np.asarray(adj_matrix) > 0).T.astype(np.uint8)
    adjT_u8 = np.ascontiguousarray(adjT_u8)

    has_bias = bool(np.any(b != 0.0))
    el_imm = float(a_bias) + float(b @ a_l)
    er_imm = float(b @ a_r)
    brow = np.ascontiguousarray(b.reshape(1, D))

    in_maps = []
    for c in range(CORES):
        qr = slice(c * QL, (c + 1) * QL)
        in_maps.append(
            {
                "xT": xT,
                "xTq": np.ascontiguousarray(xT[:, qr]),
                "waug": waug,
                "wl": wl,
                "adjT": np.ascontiguousarray(adjT_u8[:, qr]),
                "brow": brow,
            }
        )
    return in_maps, el_imm, er_imm, has_bias


def kernel(**inputs):
    adj_matrix = inputs["adj_matrix"]
    node_features = inputs["node_features"]
    W = inputs["W"]
    b = inputs["b"]
    a = inputs["a"]
    a_bias = inputs["a_bias"]

    in_maps, el_imm, er_imm, has_bias = _prep_inputs(
        adj_matrix, node_features, W, b, a, a_bias
    )
    nc = _get_program(el_imm, er_imm, has_bias)
    results = _run(nc, in_maps)
    out = np.concatenate([results[c]["hp"] for c in range(CORES)], axis=0)
    return out.astype(np.float32)


if __name__ == "__main__":
    rng = np.random.default_rng(0)
    demo = {
        "adj_matrix": rng.integers(0, 2, (N, N)).astype(np.int32),
        "node_features": rng.standard_normal((N, IN_DIM), dtype=np.float32),
        "W": rng.standard_normal((IN_DIM, D), dtype=np.float32) * 0.05,
        "b": np.zeros(D, np.float32),
        "a": rng.standard_normal(2 * D, dtype=np.float32) * 0.1,
        "a_bias": np.zeros((), np.float32),
    }
    out = kernel(**demo)
    print(out.shape, out.dtype)


# revision 11
# speedup vs baseline: 1.1930x; 1.1330x over previous
"""GAT layer kernel for Trainium2, sharded across 8 NeuronCores.

Strategy (per sharding hint): row-shard the 8192x8192 attention matrix —
core c owns query rows [c*1024, (c+1)*1024). Each core:
  Phase A: computes the full h = X @ W (8192x256, replicated) on PE, with
           e_r = h @ a_r fused in as an extra output column, plus a ones
           column appended for the softmax denominator.
  Phase B: computes e_l for its own 1024 queries (tiny matmuls), transposes
           it to a row, and broadcasts across partitions.
  Phase C: for each 128-key tile: builds unnormalized attention
           p[k, q] = exp(leakyrelu(e_l[q] + e_r[k])) * adj[q, k] directly in
           [key, query] layout (exp(leaky(t)) == max(exp(t), exp(0.2 t))),
           then accumulates h_prime^T contributions via PE matmuls
           out[q, :] += p[:, q].T @ [h | 1].  The ones column yields the
           softmax denominator for free; the final [128, 256] tile is scaled
           by its reciprocal.

Host-side prep is layout only: X transposed, adj transposed+packed to uint8
per-core slices, and the attention projection folded into the weight matrix
(waug = [W | W @ a_r], wl = W @ a_l).
"""

import numpy as np
from contextlib import ExitStack

N = 8192
IN_DIM = 512
D = 256
CORES = 8
QL = N // CORES          # queries per core
KT = N // 128            # 64 key tiles
QT = QL // 128           # 8 query tiles per core
KK = IN_DIM // 128       # 4 contraction tiles for h
HS = D + 2               # h storage stride: [h(256) | ones | pad] (fp32r needs even free sizes)
ALPHA = 0.2

_CACHE = {}


def _build_program(el_imm, er_imm, has_bias, reps=None, use_lrelu=False, pool_mult=False, ew_bf16=True, cast_pool=False, in_bf16=True):
    """Build + compile the Bass program. el_imm/er_imm are scalar offsets
    folded into e_l / e_r (a_bias + b@a_l, b@a_r). has_bias adds h += b."""
    import concourse.mybir as mybir
    import concourse.tile as tile
    from concourse import bacc
    from concourse.masks import make_identity

    f32 = mybir.dt.float32
    f32r = mybir.dt.float32r
    bf16 = mybir.dt.bfloat16
    u8 = mybir.dt.uint8
    AF = mybir.ActivationFunctionType
    ALU = mybir.AluOpType

    nc = bacc.Bacc(trn_type="TRN2", debug=False)

    idt = bf16 if in_bf16 else f32r
    xT = nc.dram_tensor("xT", [IN_DIM, N], idt, kind="ExternalInput")
    xTq = nc.dram_tensor("xTq", [IN_DIM, QL], idt, kind="ExternalInput")
    waug = nc.dram_tensor("waug", [IN_DIM, HS], idt, kind="ExternalInput")
    wl = nc.dram_tensor("wl", [IN_DIM, 2], idt, kind="ExternalInput")
    adjT = nc.dram_tensor("adjT", [N, QL], u8, kind="ExternalInput")
    brow = nc.dram_tensor("brow", [1, D], f32, kind="ExternalInput")
    hp = nc.dram_tensor("hp", [QL, D], f32, kind="ExternalOutput")

    with tile.TileContext(nc) as tc, ExitStack() as ctx:
        if reps is not None:
            ctx.enter_context(tc.For_i(0, reps, 1))
        const = ctx.enter_context(tc.tile_pool(name="const", bufs=1))

        ident = const.tile([128, 128], f32)
        make_identity(nc, ident[:])

        # waug [512, 257] -> sbuf [128, (kk, 257)]
        waug_sb = const.tile([128, KK * HS], idt)
        nc.sync.dma_start(
            waug_sb[:].rearrange("p (k n) -> p k n", n=HS),
            waug.ap().rearrange("(k p) n -> p k n", p=128),
        )
        # wl [512, 1] -> sbuf [128, 4]
        wl_sb = const.tile([128, KK * 2], idt)
        nc.sync.dma_start(
            wl_sb[:].rearrange("p (k n) -> p k n", n=2),
            wl.ap().rearrange("(k p) n -> p k n", p=128),
        )
        # xTq [512, 1024] -> sbuf [128, (kk, 1024)]
        xq = const.tile([128, KK * QL], idt)
        nc.sync.dma_start(
            xq[:].rearrange("p (k n) -> p k n", n=QL),
            xTq.ap().rearrange("(k p) n -> p k n", p=128),
        )
        if has_bias:
            brow_sb = const.tile([1, D], f32)
            nc.sync.dma_start(brow_sb[:1, :], brow.ap())
            b_bcast = const.tile([128, D], f32)
            nc.gpsimd.partition_broadcast(b_bcast[:], brow_sb[:1, :])

        # persistent state
        hdt = bf16 if ew_bf16 else f32r
        h_sb = const.tile([128, KT * HS], hdt)      # [h | 1] per key tile
        er_sb = const.tile([128, KT], f32)
        er02_sb = const.tile([128, KT], f32)
        el_own = const.tile([128, QT], f32)
        el_row = const.tile([1, QL], f32)
        el_bcast = const.tile([128, QL], f32)

        # ones columns of h storage (via f32 const copy: memset on f32r is
        # rejected by walrus, and the f32r matmul input needs a "rounding"
        # producer instruction)
        ones_src = const.tile([128, KT], f32)
        nc.vector.memset(ones_src[:], 1.0)
        zeros_src = const.tile([128, KT], f32)
        nc.vector.memset(zeros_src[:], 0.0)
        hv = h_sb[:].rearrange("p (k j) -> p k j", j=HS)
        nc.vector.tensor_copy(hv[:, :, D : D + 1], ones_src[:].rearrange("p (k n) -> p k n", n=1))
        nc.vector.tensor_copy(hv[:, :, D + 1 : D + 2], zeros_src[:].rearrange("p (k n) -> p k n", n=1))

        # ---- Phase B part 1: e_l for this core's queries (tiny matmuls) ----
        with tc.tile_pool(name="psel", bufs=1, space="PSUM") as psel_pool:
            ps_el = psel_pool.tile([128, 2 * QT], f32)
            xqv = xq[:].rearrange("p (k n) -> p k n", n=QL)
            for t in range(QT):
                for kk in range(KK):
                    nc.tensor.matmul(
                        ps_el[:, 2 * t : 2 * t + 2],
                        xqv[:, kk, t * 128 : (t + 1) * 128],
                        wl_sb[:, 2 * kk : 2 * kk + 2],
                        start=(kk == 0),
                        stop=(kk == KK - 1),
                    )
            nc.vector.tensor_copy(
                el_own[:], ps_el[:].rearrange("p (t j) -> p t j", j=2)[:, :, 0]
            )

        with tc.tile_pool(name="pselr", bufs=1, space="PSUM") as pselr_pool:
            el_row_ps = pselr_pool.tile([1, QL], f32)
            for t in range(QT):
                nc.tensor.transpose(
                    el_row_ps[0:1, t * 128 : (t + 1) * 128],
                    el_own[:, t : t + 1],
                    ident[:],
                )
            nc.vector.tensor_scalar_add(el_row[:1, :], el_row_ps[0:1, :], float(el_imm))
        nc.gpsimd.partition_broadcast(el_bcast[:], el_row[:1, :])

        # ---- Phase A: h = X @ waug over all 64 node tiles ----
        xTv = xT.ap().rearrange("(k p) n -> p k n", p=128)
        NCH = 8            # node tiles per x chunk
        with (
            tc.tile_pool(name="xch", bufs=2) as xch_pool,
            tc.tile_pool(name="psA", bufs=4, space="PSUM") as psA_pool,
        ):
            for ch in range(KT // NCH):
                xch = xch_pool.tile([128, KK * NCH * 128], idt)
                nc.sync.dma_start(
                    xch[:].rearrange("p (k n) -> p k n", n=NCH * 128),
                    xTv[:, :, ch * NCH * 128 : (ch + 1) * NCH * 128],
                )
                xchv = xch[:].rearrange("p (k n) -> p k n", n=NCH * 128)
                for t in range(NCH):
                    i = ch * NCH + t
                    psA = psA_pool.tile([128, HS], f32)
                    for kk in range(KK):
                        nc.tensor.matmul(
                            psA[:],
                            xchv[:, kk, t * 128 : (t + 1) * 128],
                            waug_sb[:, kk * HS : (kk + 1) * HS],
                            start=(kk == 0),
                            stop=(kk == KK - 1),
                        )
                    if has_bias:
                        nc.vector.tensor_tensor(
                            h_sb[:, i * HS : i * HS + D], psA[:, 0:D], b_bcast[:],
                            op=ALU.add,
                        )
                    else:
                        nc.vector.tensor_copy(h_sb[:, i * HS : i * HS + D], psA[:, 0:D])
                    nc.vector.tensor_scalar_add(
                        er_sb[:, i : i + 1], psA[:, D : D + 1], float(er_imm)
                    )
                    nc.vector.tensor_scalar(
                        er02_sb[:, i : i + 1], psA[:, D : D + 1],
                        float(er_imm), ALPHA, ALU.add, ALU.mult,
                    )

        # ---- Phase C: masked attention + h_prime accumulation ----
        with (
            tc.tile_pool(name="psC", bufs=1, space="PSUM") as psC_pool,
            tc.tile_pool(name="adjp", bufs=4) as adj_pool,
            tc.tile_pool(name="up", bufs=2) as u_pool,
            tc.tile_pool(name="vp", bufs=2) as v_pool,
            tc.tile_pool(name="sp", bufs=2) as s_pool,
            tc.tile_pool(name="pp", bufs=3) as p_pool,
            tc.tile_pool(name="abp", bufs=2) as adjb_pool,
            tc.tile_pool(name="op", bufs=2) as o_pool,
        ):
            acc = [
                psC_pool.tile([128, HS], f32, name=f"acc{jj}", tag=f"acc{jj}")
                for jj in range(QT)
            ]
            ewdt = bf16 if ew_bf16 else f32
            for k in range(KT):
                adj_t = adj_pool.tile([128, QL], u8)
                nc.sync.dma_start(adj_t[:], adjT.ap()[k * 128 : (k + 1) * 128, :])
                u_t = u_pool.tile([128, QL], ewdt)
                nc.scalar.activation(
                    u_t[:], el_bcast[:], AF.Exp, bias=er_sb[:, k : k + 1]
                )
                v_t = v_pool.tile([128, QL], ewdt)
                nc.scalar.activation(
                    v_t[:], el_bcast[:], AF.Exp, bias=er02_sb[:, k : k + 1],
                    scale=ALPHA,
                )
                s_t = s_pool.tile([128, QL], ewdt)
                nc.vector.tensor_tensor(s_t[:], u_t[:], v_t[:], op=ALU.max)
                p_t = p_pool.tile([128, QL], bf16 if ew_bf16 else f32r)
                if ew_bf16:
                    adjb = adjb_pool.tile([128, QL], bf16)
                    if cast_pool:
                        nc.gpsimd.tensor_copy(adjb[:], adj_t[:])
                    else:
                        nc.vector.tensor_copy(adjb[:], adj_t[:])
                    nc.vector.tensor_tensor(p_t[:], s_t[:], adjb[:], op=ALU.mult)
                else:
                    nc.vector.tensor_tensor(p_t[:], s_t[:], adj_t[:], op=ALU.mult)
                hk = h_sb[:, k * HS : (k + 1) * HS]
                for jj in range(QT):
                    nc.tensor.matmul(
                        acc[jj][:],
                        p_t[:, jj * 128 : (jj + 1) * 128],
                        hk,
                        start=(k == 0),
                        stop=(k == KT - 1),
                    )
            for jj in range(QT):
                recip = o_pool.tile([128, 1], f32, tag="recip")
                nc.vector.reciprocal(recip[:], acc[jj][:, D : D + 1])
                o_t = o_pool.tile([128, D], f32, tag="out")
                nc.vector.tensor_scalar_mul(o_t[:], acc[jj][:, 0:D], recip[:])
                nc.sync.dma_start(hp.ap()[jj * 128 : (jj + 1) * 128, :], o_t[:])

    nc.compile()
    return nc


def _get_program(el_imm, er_imm, has_bias, reps=None, **kw):
    key = (round(float(el_imm), 12), round(float(er_imm), 12), bool(has_bias), reps,
           tuple(sorted(kw.items())))
    if key not in _CACHE:
        _CACHE[key] = _build_program(el_imm, er_imm, has_bias, reps=reps, **kw)
    return _CACHE[key]


def _run(nc, in_maps):
    from concourse.bass_utils import run_bass_kernel_spmd

    res = run_bass_kernel_spmd(nc, in_maps, core_ids=list(range(CORES)))
    return res.results


def _prep_inputs(adj_matrix, node_features, W, b, a, a_bias, in_bf16=True):
    X = np.asarray(node_features, dtype=np.float32)
    W = np.asarray(W, dtype=np.float32)
    b = np.asarray(b, dtype=np.float32)
    a = np.asarray(a, dtype=np.float32)
    a_l, a_r = a[:D], a[D:]

    wl = np.ascontiguousarray(
        np.concatenate([(W @ a_l)[:, None], np.zeros((IN_DIM, 1), np.float32)], axis=1)
    )
    waug = np.ascontiguousarray(
        np.concatenate([W, (W @ a_r)[:, None], np.zeros((IN_DIM, 1), np.float32)], axis=1)
    )
    xT = np.ascontiguousarray(X.T)
    adjT_u8 = (np.asarray(adj_matrix) > 0).T.astype(np.uint8)
    adjT_u8 = np.ascontiguousarray(adjT_u8)

    has_bias = bool(np.any(b != 0.0))
    el_imm = float(a_bias) + float(b @ a_l)
    er_imm = float(b @ a_r)
    brow = np.ascontiguousarray(b.reshape(1, D))

    if in_bf16:
        import ml_dtypes

        bf = ml_dtypes.bfloat16
        xT = xT.astype(bf)
        waug = waug.astype(bf)
        wl = wl.astype(bf)

    in_maps = []
    for c in range(CORES):
        qr = slice(c * QL, (c + 1) * QL)
        in_maps.append(
            {
                "xT": xT,
                "xTq": np.ascontiguousarray(xT[:, qr]),
                "waug": waug,
                "wl": wl,
                "adjT": np.ascontiguousarray(adjT_u8[:, qr]),
                "brow": brow,
            }
        )
    return in_maps, el_imm, er_imm, has_bias


def kernel(**inputs):
    adj_matrix = inputs["adj_matrix"]
    node_features = inputs["node_features"]
    W = inputs["W"]
    b = inputs["b"]
    a = inputs["a"]
    a_bias = inputs["a_bias"]

    in_maps, el_imm, er_imm, has_bias = _prep_inputs(
        adj_matrix, node_features, W, b, a, a_bias
    )
    nc = _get_program(el_imm, er_imm, has_bias)
    results = _run(nc, in_maps)
    out = np.concatenate([results[c]["hp"] for c in range(CORES)], axis=0)
    return out.astype(np.float32)


if __name__ == "__main__":
    rng = np.random.default_rng(0)
    demo = {
        "adj_matrix": rng.integers(0, 2, (N, N)).astype(np.int32),
        "node_features": rng.standard_normal((N, IN_DIM), dtype=np.float32),
        "W": rng.standard_normal((IN_DIM, D), dtype=np.float32) * 0.05,
        "b": np.zeros(D, np.float32),
        "a": rng.standard_normal(2 * D, dtype=np.float32) * 0.1,
        "a_bias": np.zeros((), np.float32),
    }
    out = kernel(**demo)
    print(out.shape, out.dtype)
